# revision 59
# baseline (speedup 1.0000x reference)
"""Trainium2 Bass kernel for nn_DPRNN_TAC (DPRNN + TAC, L=2 layers). v2

Sharding: one (batch, channel) pair per NeuronCore (B*CH = 8 = n_cores).
Row/col BiLSTMs, group norms and TAC MLPs are core-local; the TAC channel
mean is a ReduceScatter (masked ch, bf16) per layer; the concat-MLP's
cm-dependent half is computed on the RS shard and AllGathered as a small
[N, pos/4] tensor.

v2 changes vs v1:
 - The residual stream lives in SBUF as bf16 (no HBM round trips per
   phase; applies are 2 DVE ops per chunk).
 - cc_w is split: z_cc1 = ccw[:, :E] @ ch computed in the tr phase
   (PE, overlapped), spilled bf16 to DRAM; z_cc2 = ccw[:, E:] @ av(cm)
   computed on the RS shard and AllGathered (6x smaller collective).
 - LSTM scan: tanh outputs / cell state / h in bf16 (2x DVE modes);
   the i*g product runs on GPSIMD; all gate math via tanh only
   (sigmoid(x) = 0.5 + 0.5*tanh(x/2), scales folded host-side).
Precision: validated in a numpy golden model, rel err ~7.6e-3 (vs
tolerance 2e-2).
"""

import numpy as np
import ml_dtypes

BF16 = ml_dtypes.bfloat16

L, B, CH, N, H, D1, D2, OUT = 2, 2, 4, 64, 128, 100, 200, 64
E = 3 * H
NCORES = 8

_CACHE = {}


def prep_weights(inp, core_b, core_c, layers=L):
    """Host-side weight transforms for one core. Returns dict[str, np.ndarray]."""
    # gate order reshuffled to (i, f, o, g); i/f/o halved (sigmoid via
    # scale=2 at the ACT), g full-scale (tanh); h and c stored directly.
    perm = [0, 1, 3, 2]
    gs = np.array([0.5, 0.5, 0.5, 1.0], np.float32)  # scales for i,f,o,g
    num_mic = np.asarray(inp["num_mic"]).astype(np.int64)
    eff = int(num_mic[core_b]) if int(num_mic.max()) > 0 else CH
    if eff <= 0:
        eff = CH
    w = {}
    scan_bias_nonzero = False
    for r in ("row", "col"):
        for i in range(layers):
            bsum = np.asarray(inp[f"{r}_bih"][i]) + np.asarray(inp[f"{r}_bhh"][i])
            if np.any(np.asarray(bsum) != 0):
                scan_bias_nonzero = True
    w["_has_xbias"] = scan_bias_nonzero

    for r in ("row", "col"):
        for i in range(layers):
            whh = np.zeros((2, 4, H, H), np.float32)
            wx4 = np.zeros((2, 4, N, H), np.float32)
            gb = np.zeros((2, 4, H), np.float32)
            for d in range(2):
                Wih = np.asarray(inp[f"{r}_Wih"][i][d], np.float32).reshape(4, H, N)
                Whh = np.asarray(inp[f"{r}_Whh"][i][d], np.float32).reshape(4, H, H)
                bsum = (
                    np.asarray(inp[f"{r}_bih"][i][d], np.float32)
                    + np.asarray(inp[f"{r}_bhh"][i][d], np.float32)
                ).reshape(4, H)
                for g in range(4):
                    src = perm[g]
                    whh[d, g] = (Whh[src] * gs[g]).T  # lhsT [h, gh]
                    wx4[d, g] = (Wih[src] * gs[g]).T  # [N, H]
                    gb[d, g] = bsum[src]  # raw; ACT bias is pre-nonlinearity
            # SBUF layout: partition dim first
            w[f"whh_{r}{i}"] = np.moveaxis(whh, 2, 0).astype(BF16)  # [H,2,4,H]
            w[f"wxb_{r}{i}"] = np.moveaxis(wx4, 2, 0).astype(BF16)  # [N,2,4,H]
            if scan_bias_nonzero:
                w[f"gb_{r}{i}"] = np.moveaxis(gb, 2, 0).copy()  # [H,2,4]
            pw = np.asarray(inp[f"{r}_proj_w"][i], np.float32)  # [N, 2H]
            pj = np.zeros((2, H, N), np.float32)
            pj[0] = pw[:, :H].T
            pj[1] = pw[:, H:].T
            w[f"proj_{r}{i}"] = np.moveaxis(pj, 1, 0).astype(BF16)  # [H,2,N]
            w[f"pb_{r}{i}"] = np.asarray(
                inp[f"{r}_proj_b"][i], np.float32).reshape(N, 1)
    for i in range(layers):
        trw = np.asarray(inp["tr_w"][i], np.float32)  # [E, N]
        w[f"trw{i}"] = np.ascontiguousarray(trw.T.reshape(N, 3, H)).astype(BF16)
        w[f"trb{i}"] = np.ascontiguousarray(
            np.asarray(inp["tr_b"][i], np.float32).reshape(3, H).T)  # [H,3]
        avw = np.asarray(inp["av_w"][i], np.float32) / float(eff)  # [E, E]
        avw4 = np.ascontiguousarray(avw.T.reshape(3, H, 3, H))  # [kt,k,mt,m]
        w[f"avw{i}"] = np.moveaxis(avw4, 1, 0).astype(BF16)  # [H,kt,mt,m]
        w[f"avb{i}"] = np.ascontiguousarray(
            np.asarray(inp["av_b"][i], np.float32).reshape(3, H).T)  # [H,3]
        ccw = np.asarray(inp["cc_w"][i], np.float32)  # [N, 2E]
        ccw6 = np.ascontiguousarray(ccw.T.reshape(6, H, N))
        ccws = np.moveaxis(ccw6, 1, 0).astype(BF16)  # [H,6,N]
        w[f"ccw1_{i}"] = np.ascontiguousarray(ccws[:, 0:3])  # [H,3,N]
        w[f"ccw2_{i}"] = np.ascontiguousarray(ccws[:, 3:6])  # [H,3,N]
        w[f"ccb{i}"] = np.asarray(inp["cc_b"][i], np.float32).reshape(N, 1)
        for nm in ("rn", "cn", "chn"):
            w[f"{nm}w{i}"] = np.asarray(inp[f"{nm}_w"][i], np.float32).reshape(N, 1)
            w[f"{nm}b{i}"] = np.asarray(inp[f"{nm}_b"][i], np.float32).reshape(N, 1)
    w["outw"] = np.ascontiguousarray(
        np.asarray(inp["out_w"], np.float32).T).astype(BF16)  # [N, OUT]
    w["outb"] = np.asarray(inp["out_b"], np.float32).reshape(OUT, 1)
    w["msk"] = np.full((H, 1), 1.0 if core_c < eff else 0.0, np.float32)
    return w


def build_program(has_xbias, alphas, bias_flags, d1=D1, d2=D2, layers=L,
                  n_cores=NCORES, n_iter=1, with_cc=True, nq=2):
    import concourse.bass as bass
    import concourse.tile as tile
    from concourse import bacc, mybir
    import contextlib

    f32 = mybir.dt.float32
    bf16 = mybir.dt.bfloat16
    fp8 = mybir.dt.float8e4
    AF = mybir.ActivationFunctionType
    OP = mybir.AluOpType

    pos = d1 * d2
    assert pos % 4 == 0
    blk = pos // 4  # allreduce block per group rank
    cw = next(c for c in (500, 512, 400, 256, 200, 128, 100, 64, 50, 32, 20)
              if pos % c == 0)
    nch = pos // cw
    cwl = next(c for c in (1000, 500, cw) if pos % c == 0)
    nchl = pos // cwl
    # cn-apply chunk: whole p-rows, <=512
    rows_per = max(1, 400 // d2) if d2 <= 400 else 1
    cwc = rows_per * d2
    while pos % cwc != 0:
        rows_per -= 1
        cwc = rows_per * d2
    nchc = pos // cwc
    tr_a, av_a, cc_a, out_a = alphas
    tr_bnz, av_bnz, cc_bnz = bias_flags
    n_groups = n_cores // 4
    rgroups = [[g * 4 + j for j in range(4)] for g in range(n_groups)]

    nc = bacc.Bacc("TRN2", target_bir_lowering=False, debug=False,
                   num_devices=n_cores)

    x_in = nc.dram_tensor("x", [N, pos], f32, kind="ExternalInput")
    y_out = nc.dram_tensor("y", [OUT, pos], f32, kind="ExternalOutput")

    def din(name, shape, dt):
        return nc.dram_tensor(name, shape, dt, kind="ExternalInput")

    wt = {}
    for r in ("row", "col"):
        for i in range(layers):
            wt[f"whh_{r}{i}"] = din(f"whh_{r}{i}", [H, 2, 4, H], bf16)
            wt[f"wxb_{r}{i}"] = din(f"wxb_{r}{i}", [N, 2, 4, H], bf16)
            if has_xbias:
                wt[f"gb_{r}{i}"] = din(f"gb_{r}{i}", [H, 2, 4], f32)
            wt[f"proj_{r}{i}"] = din(f"proj_{r}{i}", [H, 2, N], bf16)
            wt[f"pb_{r}{i}"] = din(f"pb_{r}{i}", [N, 1], f32)
    for i in range(layers):
        wt[f"trw{i}"] = din(f"trw{i}", [N, 3, H], bf16)
        wt[f"trb{i}"] = din(f"trb{i}", [H, 3], f32)
        wt[f"avw{i}"] = din(f"avw{i}", [H, 3, 3, H], bf16)
        wt[f"avb{i}"] = din(f"avb{i}", [H, 3], f32)
        wt[f"ccw1_{i}"] = din(f"ccw1_{i}", [H, 3, N], bf16)
        wt[f"ccw2_{i}"] = din(f"ccw2_{i}", [H, 3, N], bf16)
        wt[f"ccb{i}"] = din(f"ccb{i}", [N, 1], f32)
        for nm in ("rn", "cn", "chn"):
            wt[f"{nm}w{i}"] = din(f"{nm}w{i}", [N, 1], f32)
            wt[f"{nm}b{i}"] = din(f"{nm}b{i}", [N, 1], f32)
    wt["outw"] = din("outw", [N, OUT], bf16)
    wt["outb"] = din("outb", [OUT, 1], f32)
    wt["msk"] = din("msk", [H, 1], f32)

    with tile.TileContext(nc) as tc:
        with contextlib.ExitStack() as ctx:
            singles = ctx.enter_context(tc.tile_pool(name="singles", bufs=1))
            xz = ctx.enter_context(tc.tile_pool(name="xz", bufs=2))
            tmp = ctx.enter_context(tc.tile_pool(name="tmp", bufs=2))
            chk = ctx.enter_context(tc.tile_pool(name="chk", bufs=2))
            dram = ctx.enter_context(tc.tile_pool(name="dram", bufs=1, space="DRAM"))

            sw = {}
            for k, t in wt.items():
                sw[k] = singles.tile(list(t.shape), t.dtype, tag=f"w_{k}", name=f"sw_{k}")
                nc.sync.dma_start(out=sw[k][:], in_=t[:])

            # residual stream, SBUF-resident bf16
            out_sb = singles.tile([N, pos], bf16, tag="out_sb", name="out_sb")

            zcc1_dram = dram.tile([N, pos], bf16)
            # Variable-size strips (decreasing): early strips are big so
            # their ReduceScatter overlaps the rest of the tr phase; the
            # last strip is small so the RS->av->AG->cc tail is short.
            NQ = nq
            frac = {1: [1.0], 2: [0.7, 0.3], 3: [0.5, 0.3, 0.2],
                    4: [0.4, 0.3, 0.2, 0.1]}[NQ]
            ssz = [max(250, int(blk * f / 250) * 250) for f in frac]
            ssz[-1] = blk - sum(ssz[:-1])
            assert all(s > 0 and s % 250 == 0 for s in ssz), ssz
            soff = [sum(ssz[:j]) for j in range(NQ)]
            bounce_in_s = [dram.tile([4, 3, H, ssz[j]], fp8, name=f"bin{j}",
                                     tag=f"bin{j}") for j in range(NQ)]
            bounce_rs_s = [dram.tile([3, H, ssz[j]], fp8, name=f"brs{j}",
                                     tag=f"brs{j}") for j in range(NQ)]
            zin_s = [dram.tile([N, ssz[j]], bf16, name=f"zin{j}",
                               tag=f"zin{j}") for j in range(NQ)]
            zout_s = [dram.tile([4, N, ssz[j]], bf16, name=f"zout{j}",
                                tag=f"zout{j}") for j in range(NQ)]

            def strip_of(qo):
                for j in range(NQ - 1, -1, -1):
                    if qo >= soff[j]:
                        return j
                return 0
            gn_dram = dram.tile([1, 2], f32)

            def new_z():
                return xz.tile([N, pos], bf16, tag="xz", name="zt")

            ones_k = singles.tile([N, 1], f32, tag="ones_k")
            nc.vector.memset(ones_k[:], 1.0)
            epsc = singles.tile([1, 1], f32, tag="epsc")
            nc.vector.memset(epsc[:], 1e-8)

            def gn_finalize(stats, wv, bv, ps):
                """stats [N, nchunks, 6] -> per-partition s,t [N,1] f32."""
                mv = tmp.tile([N, 2], f32, tag="gnmv")
                nc.vector.bn_aggr(out=mv[:], in_=stats[:])
                r3 = tmp.tile([N, 3], f32, tag="gnr3")
                nc.vector.tensor_copy(r3[:, 0:2], mv[:, 0:2])
                nc.vector.tensor_mul(r3[:, 2:3], mv[:, 0:1], mv[:, 0:1])
                pr = ps.tile([128, 512], f32, tag="ps1")
                nc.tensor.matmul(pr[0:1, 0:3], ones_k[:], r3[:],
                                 start=True, stop=True)
                sc = tmp.tile([1, 3], f32, tag="gnsc")
                nc.vector.tensor_scalar_mul(sc[0:1, 0:3], pr[0:1, 0:3], 1.0 / N)
                var = tmp.tile([1, 1], f32, tag="gnvar")
                nc.vector.tensor_mul(var[0:1], sc[0:1, 0:1], sc[0:1, 0:1])
                nc.vector.tensor_sub(var[0:1], sc[0:1, 1:2], var[0:1])
                nc.vector.tensor_add(var[0:1], var[0:1], sc[0:1, 2:3])
                rm = tmp.tile([1, 2], f32, tag="gnrm")
                nc.scalar.activation(out=rm[0:1, 0:1], in_=var[0:1], func=AF.Sqrt,
                                     bias=epsc[0:1, :], scale=1.0)
                nc.vector.reciprocal(rm[0:1, 0:1], rm[0:1, 0:1])
                nc.vector.tensor_scalar_mul(rm[0:1, 1:2], sc[0:1, 0:1], -1.0)
                rb = tmp.tile([N, 2], f32, tag="gnrb")
                nc.gpsimd.partition_broadcast(rb[:], rm[0:1, :])
                s = tmp.tile([N, 1], f32, tag="gns")
                t = tmp.tile([N, 1], f32, tag="gnt")
                nc.vector.tensor_mul(s[:], wv[:], rb[:, 0:1])
                nc.vector.scalar_tensor_tensor(
                    out=t[:], in0=s[:], scalar=rb[:, 1:2], in1=bv[:],
                    op0=OP.mult, op1=OP.add)
                return s, t

            # ---------------- LSTM scan (proj fused) ----------------
            def scan(r, i, T, M, x_ap):
                """Interleaved fwd/bwd scan with the 2H->N projection fused in
                (lagged one step). x_ap(d, k) -> AP [N, M] (bf16). Writes z
                (bf16 SBUF, seq-major blocks of M) and returns it."""
                assert T % 2 == 0
                z = new_z()
                whh = sw[f"whh_{r}{i}"]
                wxb = sw[f"wxb_{r}{i}"]
                pj = sw[f"proj_{r}{i}"]
                pb = sw[f"pb_{r}{i}"]
                gb = sw.get(f"gb_{r}{i}")
                PW = 256 if M > 128 else 128
                wst = tmp.tile([H, 2, M], bf16, tag="wst")
                nc.vector.memset(wst[:], 0.0)
                hr = tmp.tile([H, 2, 2, M], bf16, tag="hr")  # [h, k%2, dir, M]

                with tc.tile_pool(name="psg", bufs=1, space="PSUM") as psg, \
                        tc.tile_pool(name="psp", bufs=2, space="PSUM") as psp:

                    def proj(kp):
                        """z += hr(kp) @ proj, both dirs; bias on first touch."""
                        pps = []
                        for d in range(2):
                            pp = psp.tile([128, PW], f32, tag="pp")
                            nc.tensor.matmul(pp[0:N, 0:M], pj[:, d, :],
                                             hr[:, kp % 2, d, :],
                                             start=True, stop=True)
                            pps.append(pp)
                        for d in range(2):
                            seq = kp if d == 0 else (T - 1 - kp)
                            sl = slice(seq * M, (seq + 1) * M)
                            if 2 * kp < T:
                                nc.vector.tensor_scalar_add(
                                    z[:, sl], pps[d][0:N, 0:M], pb[:])
                            else:
                                nc.vector.tensor_add(
                                    z[:, sl], z[:, sl], pps[d][0:N, 0:M])

                    psts = [None, None]
                    for k in range(T):
                        for d in range(2):
                            pst = psg.tile([128, 4, PW], f32, tag=f"psg{d}")
                            psts[d] = pst
                            for g in range(4):
                                nc.tensor.matmul(
                                    pst[:, g, 0:M], wxb[:, d, g, :], x_ap(d, k),
                                    start=True, stop=(k == 0))
                                if k > 0:
                                    nc.tensor.matmul(
                                        pst[:, g, 0:M], whh[:, d, g, :],
                                        hr[:, (k - 1) % 2, d, :],
                                        start=False, stop=True)
                        if k > 0:
                            proj(k - 1)
                        # gates order (i, f, o, g): i/f/o sigmoid (scale=2
                        # undoes the halved weights), g tanh
                        Sts = [None, None]
                        Tgs = [None, None]
                        for d in range(2):
                            St = tmp.tile([H, 3, M], bf16, tag=f"St{d}")
                            Sts[d] = St
                            Tg = tmp.tile([H, M], bf16, tag=f"Tg{d}")
                            Tgs[d] = Tg
                            if gb is None:
                                nc.scalar.activation(out=St[:],
                                                     in_=psts[d][:, 0:3, 0:M],
                                                     func=AF.Sigmoid, scale=2.0)
                                nc.scalar.activation(out=Tg[:],
                                                     in_=psts[d][:, 3, 0:M],
                                                     func=AF.Tanh)
                            else:
                                for g in range(3):
                                    nc.scalar.activation(
                                        out=St[:, g, :], in_=psts[d][:, g, 0:M],
                                        func=AF.Sigmoid, scale=2.0,
                                        bias=gb[:, d, g:g + 1])
                                nc.scalar.activation(
                                    out=Tg[:], in_=psts[d][:, 3, 0:M],
                                    func=AF.Tanh, bias=gb[:, d, 3:4])
                        for d in range(2):
                            # pt = sig_f * c
                            pt = tmp.tile([H, M], bf16, tag=f"pt{d}")
                            nc.vector.tensor_mul(pt[:], Sts[d][:, 1, :],
                                                 wst[:, d, :])
                            # qt = sig_i * tanh_g
                            qt = tmp.tile([H, M], bf16, tag=f"qt{d}")
                            nc.vector.tensor_mul(qt[:], Sts[d][:, 0, :], Tgs[d][:])
                            # c' = pt + qt
                            nc.vector.tensor_add(wst[:, d, :], pt[:], qt[:])
                        for d in range(2):
                            Tc = tmp.tile([H, M], bf16, tag=f"Tc{d}")
                            nc.scalar.activation(out=Tc[:], in_=wst[:, d, :],
                                                 func=AF.Tanh)
                            # h = sig_o * tanh(c)
                            nc.vector.tensor_mul(hr[:, k % 2, d, :],
                                                 Sts[d][:, 2, :], Tc[:])
                    proj(T - 1)
                return z

            # ------------- GN stats over SBUF z -------------
            def stats_gn(z, gnw, gnb, ps, seq=None):
                stats = tmp.tile([N, nch, 6], f32, tag="gnstats")
                cis = list(range(nch))
                if seq is not None:
                    # z is written seq-major by two directions; issue stats
                    # for chunks in readiness order (middle-out)
                    T, M = seq
                    def ready(ci):
                        fb = (ci * cw) // M
                        lb = ((ci + 1) * cw - 1) // M
                        return max(lb, T - 1 - fb)
                    cis.sort(key=ready)
                for ci in cis:
                    off = ci * cw
                    nc.vector.bn_stats(out=stats[:, ci, :], in_=z[:, off:off + cw])
                return gn_finalize(stats, gnw, gnb, ps)

            # ---- apply: out += z*s + t (SBUF, bf16) ----
            def apply_sbuf(z, s, t, z_ap=None, cwa=None, ncha=None,
                           consumer=None, order=None, hooks=None, v_eng=None):
                cwa = cwa or cw
                ncha = ncha or nch
                for ci in (order if order is not None else range(ncha)):
                    off = ci * cwa
                    v = chk.tile([N, max(cw, cwc)], bf16, tag="vt")
                    zin = z_ap(z, ci) if z_ap is not None else z[:, off:off + cwa]
                    (v_eng or nc.vector).tensor_scalar(
                        out=v[:, 0:cwa], in0=zin, scalar1=s[:], scalar2=t[:],
                        op0=OP.mult, op1=OP.add)
                    nc.vector.tensor_add(out_sb[:, off:off + cwa],
                                         out_sb[:, off:off + cwa], v[:, 0:cwa])
                    if consumer is not None:
                        consumer(ci, off)
                    if hooks is not None and ci in hooks:
                        hooks[ci]()

            def dma_split_strip(tiles, sb, off, width, write=True, eng=None):
                """DMA [H,3,width] SBUF <-> per-strip bounce tiles
                [4,3,H,sub], splitting at rank-block and strip boundaries."""
                eng = eng or nc.sync
                done = 0
                while done < width:
                    o = off + done
                    q, qo = o // blk, o % blk
                    j = strip_of(qo)
                    so = qo - soff[j]
                    wseg = min(width - done, ssz[j] - so)
                    dr = tiles[j][q, :, :, so:so + wseg].transpose([1, 0, 2])
                    if write:
                        eng.dma_start(out=dr, in_=sb[:, :, done:done + wseg])
                    else:
                        eng.dma_start(out=sb[:, :, done:done + wseg], in_=dr)
                    done += wseg

            def dma_split_strip_z(sb, off, width, eng=None):
                """DMA [N,width] SBUF <- zout strips [4,N,sub]."""
                eng = eng or nc.gpsimd
                done = 0
                while done < width:
                    o = off + done
                    q, qo = o // blk, o % blk
                    j = strip_of(qo)
                    so = qo - soff[j]
                    wseg = min(width - done, ssz[j] - so)
                    eng.dma_start(out=sb[:, done:done + wseg],
                                  in_=zout_s[j][q, :, so:so + wseg])
                    done += wseg

            def strip_batches(cwa, ncha, use_max):
                bt = [[] for _ in range(NQ)]
                for ci in range(ncha):
                    ss = [strip_of(o % blk) for o in range(ci * cwa,
                                                           (ci + 1) * cwa, 50)]
                    ss.append(strip_of(((ci + 1) * cwa - 1) % blk))
                    bt[max(ss) if use_max else min(ss)].append(ci)
                return bt

            def ilv(n):
                """[n-1, 0, n-2, 1, ...]: unblocks both scan directions of a
                following row scan as early as possible."""
                o = []
                a, b = 0, n - 1
                while a <= b:
                    if a != b:
                        o.append(b)
                    o.append(a)
                    a, b = a + 1, b - 1
                return o

            # ================= network =================
            def _network():
                # init: out_sb = bf16(x)
                for ci in ilv(nch):
                    off = ci * cw
                    xc = chk.tile([N, cw], f32, tag="xc")
                    nc.sync.dma_start(out=xc[:], in_=x_in[:, off:off + cw])
                    eng = nc.gpsimd if ci % 2 == 0 else nc.vector
                    eng.tensor_copy(out=out_sb[:, off:off + cw], in_=xc[:])

                for i in range(layers):
                    # ---- row: sequences along d1 (t=p), batch d2 ----
                    z = scan("row", i, d1, d2,
                             lambda d, k: out_sb[0:N,
                                                 (k if d == 0 else d1 - 1 - k) * d2:
                                                 (k + 1 if d == 0 else d1 - k) * d2])
                    with tc.tile_pool(name="psA", bufs=2, space="PSUM") as psA:
                        s, t = stats_gn(z, sw[f"rnw{i}"], sw[f"rnb{i}"], psA,
                                        seq=(d1, d2))
                        apply_sbuf(z, s, t)

                    # ---- col: sequences along d2 (t=q), batch d1; strided x ----
                    def xcol(d, k):
                        q = k if d == 0 else d2 - 1 - k
                        a = out_sb[0:N, q:q + 1]
                        return bass.AP(tensor=a.tensor, offset=a.offset,
                                       ap=[a.ap[0], [d2, d1]])

                    zq = scan("col", i, d2, d1, xcol)

                    # cn-apply in p-row-aligned chunks with strided z view,
                    # fused with TAC tr + mask + bounce write + z_cc1
                    def zq_ap(zt, ci, _rp=rows_per):
                        p0 = ci * _rp
                        a = zt[:, p0:p0 + 1]
                        return bass.AP(tensor=a.tensor, offset=a.offset,
                                       ap=[a.ap[0], [1, _rp], [d1, d2]])

                    trw = sw[f"trw{i}"]
                    trb = sw[f"trb{i}"]
                    msk = sw["msk"]
                    ccw1 = sw[f"ccw1_{i}"]
                    ccw2 = sw[f"ccw2_{i}"]

                    with tc.tile_pool(name="psB", bufs=2, space="PSUM") as psB:
                        s2_, t2_ = stats_gn(zq, sw[f"cnw{i}"], sw[f"cnb{i}"], psB,
                                            seq=(d2, d1))

                        def tr_consumer(ci, off, _i=i):
                            wdt = cwc
                            pp3 = psB.tile([128, 3, 512], f32, tag="pp3")
                            for e in range(3):
                                nc.tensor.matmul(
                                    pp3[:, e, 0:wdt], trw[:, e, :],
                                    out_sb[:, off:off + wdt],
                                    start=True, stop=True)
                            chc = chk.tile([H, 3, cwc], bf16, tag="chc")
                            if tr_bnz:
                                for e in range(3):
                                    nc.scalar.activation(
                                        out=chc[:, e, 0:wdt], in_=pp3[:, e, 0:wdt],
                                        func=AF.Prelu, bias=trb[:, e:e + 1],
                                        alpha=tr_a[_i])
                            else:
                                nc.scalar.activation(
                                    out=chc[:, :, 0:wdt], in_=pp3[:, :, 0:wdt],
                                    func=AF.Prelu, alpha=tr_a[_i])
                            con = chk.tile([H, 3, cwc], fp8, tag="conc")
                            nc.vector.tensor_scalar_mul(con[:, :, 0:wdt],
                                                        chc[:, :, 0:wdt], msk[:])
                            dma_split_strip(bounce_in_s, con[:, :, 0:wdt],
                                            off, wdt, eng=nc.sync)
                            # z_cc1 = ccw1 @ chc -> bf16 -> DRAM
                            ppz = psB.tile([128, 512], f32, tag="ps1")
                            for kt in range(3):
                                nc.tensor.matmul(ppz[0:N, 0:wdt], ccw1[:, kt, :],
                                                 chc[:, kt, 0:wdt],
                                                 start=(kt == 0), stop=(kt == 2))
                            zc1 = chk.tile([N, cwc], bf16, tag="zc1")
                            nc.scalar.copy(out=zc1[:, 0:wdt],
                                           in_=ppz[0:N, 0:wdt])
                            nc.scalar.dma_start(
                                out=zcc1_dram[:, off:off + wdt], in_=zc1[:, 0:wdt])

                        avw = sw[f"avw{i}"]
                        avb = sw[f"avb{i}"]
                        bw = 500

                        def av_strip(j, _i=i):
                            for bo in range(0, ssz[j], bw):
                                cmp8 = chk.tile([H, 3, bw], fp8, tag="avc8")
                                nc.gpsimd.dma_start(
                                    out=cmp8[:],
                                    in_=bounce_rs_s[j][:, :, bo:bo + bw]
                                    .transpose([1, 0, 2]))
                                cmp_ = chk.tile([H, 3, bw], bf16, tag="avc")
                                nc.scalar.copy(out=cmp_[:], in_=cmp8[:])
                                cml = chk.tile([H, 3, bw], bf16, tag="avl")
                                pp3a = psB.tile([128, 3, 512], f32, tag="pp3")
                                for mt in range(3):
                                    for kt in range(3):
                                        nc.tensor.matmul(pp3a[:, mt, 0:bw],
                                                         avw[:, kt, mt, :],
                                                         cmp_[:, kt, :],
                                                         start=(kt == 0),
                                                         stop=(kt == 2))
                                if av_bnz:
                                    for mt in range(3):
                                        nc.scalar.activation(
                                            out=cml[:, mt, :],
                                            in_=pp3a[:, mt, 0:bw],
                                            func=AF.Prelu,
                                            bias=avb[:, mt:mt + 1],
                                            alpha=av_a[_i])
                                else:
                                    nc.scalar.activation(
                                        out=cml[:], in_=pp3a[:, :, 0:bw],
                                        func=AF.Prelu, alpha=av_a[_i])
                                # z_cc2 = ccw2 @ cml -> bf16 -> zin strip
                                ppz2 = psB.tile([128, 512], f32, tag="ps1")
                                for kt in range(3):
                                    nc.tensor.matmul(ppz2[0:N, 0:bw],
                                                     ccw2[:, kt, :],
                                                     cml[:, kt, :],
                                                     start=(kt == 0),
                                                     stop=(kt == 2))
                                zc2 = chk.tile([N, bw], bf16, tag="zc2")
                                nc.scalar.copy(out=zc2[:], in_=ppz2[0:N, 0:bw])
                                nc.gpsimd.dma_start(
                                    out=zin_s[j][:, bo:bo + bw], in_=zc2[:])
                            if with_cc:
                                nc.gpsimd.collective_compute(
                                    "AllGather", OP.bypass,
                                    replica_groups=rgroups,
                                    ins=[zin_s[j].opt()],
                                    outs=[zout_s[j].opt()])

                        def rs_fire(j):
                            if with_cc:
                                nc.gpsimd.collective_compute(
                                    "ReduceScatter", OP.add,
                                    replica_groups=rgroups,
                                    ins=[bounce_in_s[j].opt()],
                                    outs=[bounce_rs_s[j].opt()])

                        bt_tr = strip_batches(cwc, nchc, use_max=False)
                        order = [ci for j in range(NQ) for ci in bt_tr[j]]
                        hooks = {}
                        for j in range(NQ):
                            def hook(_j=j):
                                rs_fire(_j)
                            hooks[bt_tr[j][-1]] = hook
                        apply_sbuf(zq, s2_, t2_, z_ap=zq_ap, cwa=cwc, ncha=nchc,
                                   consumer=tr_consumer, order=order, hooks=hooks)
                        for j in range(NQ):
                            av_strip(j)

                        # ---- cc + chn stats ----
                        ccb = sw[f"ccb{i}"]
                        z2 = new_z()
                        stats2 = tmp.tile([N, nch, 6], f32, tag="gnstats")
                        bt_cc = strip_batches(cw, nch, use_max=True)
                        for ci in (c for j in range(NQ) for c in bt_cc[j]):
                            off = ci * cw
                            zc1r = chk.tile([N, cw], bf16, tag="zc1r")
                            nc.scalar.dma_start(out=zc1r[:],
                                                in_=zcc1_dram[:, off:off + cw])
                            zc2r = chk.tile([N, cw], bf16, tag="zc2r")
                            dma_split_strip_z(zc2r[:], off, cw, eng=nc.sync)
                            x2 = chk.tile([N, cw], bf16, tag="x2")
                            nc.vector.tensor_add(x2[:], zc1r[:], zc2r[:])
                            if cc_bnz:
                                nc.scalar.activation(
                                    out=z2[:, off:off + cw], in_=x2[:],
                                    func=AF.Prelu, bias=ccb[:], alpha=cc_a[i])
                            else:
                                nc.scalar.activation(
                                    out=z2[:, off:off + cw], in_=x2[:],
                                    func=AF.Prelu, alpha=cc_a[i])
                            nc.vector.bn_stats(out=stats2[:, ci, :],
                                               in_=z2[:, off:off + cw])
                        s2, t2 = gn_finalize(stats2, sw[f"chnw{i}"],
                                             sw[f"chnb{i}"], psB)

                        if i < layers - 1:
                            apply_sbuf(z2, s2, t2, order=ilv(nch))
                        else:
                            outw = sw["outw"]
                            outb = sw["outb"]

                            def fin_consumer(ci, off):
                                yp = chk.tile([N, cw], bf16, tag="yp")
                                nc.scalar.activation(out=yp[:],
                                                     in_=out_sb[:, off:off + cw],
                                                     func=AF.Prelu,
                                                     alpha=out_a)
                                pp = psB.tile([128, 512], f32, tag="ps1")
                                nc.tensor.matmul(pp[0:OUT, 0:cw], outw[:], yp[:],
                                                 start=True, stop=True)
                                yc = chk.tile([OUT, cw], f32, tag="yc")
                                nc.vector.tensor_scalar_add(yc[:], pp[0:OUT, 0:cw],
                                                            outb[:])
                                nc.gpsimd.dma_start(out=y_out[:, off:off + cw],
                                                    in_=yc[:])

                            apply_sbuf(z2, s2, t2, cwa=cw, ncha=nch,
                                       consumer=fin_consumer, order=ilv(nch))

            for _ in range(n_iter):
                _network()

    nc.compile()
    return nc


def make_in_maps(inputs, d1=D1, d2=D2, layers=L):
    x = np.asarray(inputs["x"], np.float32)
    per_core = []
    has_xbias = None
    for b in range(B):
        for c in range(CH):
            w = prep_weights(inputs, b, c, layers)
            has_xbias = w.pop("_has_xbias")
            m = {"x": np.ascontiguousarray(x[b, c].reshape(N, d1 * d2))}
            for k, v in w.items():
                m[k] = np.ascontiguousarray(v)
            per_core.append(m)
    alphas = (
        [float(np.asarray(inputs["tr_a"][i])) for i in range(layers)],
        [float(np.asarray(inputs["av_a"][i])) for i in range(layers)],
        [float(np.asarray(inputs["cc_a"][i])) for i in range(layers)],
        float(np.asarray(inputs["out_a"])),
    )
    bias_flags = tuple(
        bool(np.any(np.asarray(inputs[k]) != 0))
        for k in ("tr_b", "av_b", "cc_b"))
    return per_core, has_xbias, alphas, bias_flags


def kernel(**inputs):
    import concourse.bass_utils as bass_utils

    per_core, has_xbias, alphas, bias_flags = make_in_maps(inputs)
    ck = (has_xbias, tuple(map(tuple, alphas[:3])), alphas[3], bias_flags)
    if ck not in _CACHE:
        _CACHE[ck] = build_program(has_xbias, alphas, bias_flags)
    nc = _CACHE[ck]
    r = bass_utils.run_bass_kernel_spmd(nc, per_core, core_ids=list(range(NCORES)))
    ys = np.zeros((B * CH, OUT, D1, D2), np.float32)
    for ci in range(NCORES):
        ys[ci] = r.results[ci]["y"].reshape(OUT, D1, D2)
    return ys
